# revision 30
# baseline (speedup 1.0000x reference)
"""Multi-head attention (16 heads, S=2048, E=1024, D=M=64, O=1024) on 8 trn2
NeuronCores, head-sharded: 2 heads per core, partial output summed on host.

v6: deadline-scheduled DMA across the three usable rings (sync/act/gpsimd
HWDGE+SWDGE, ~70 GB/s each measured): z stays bf16 but streams as 16
quarter-pieces so every t-chunk lands before the exp stream needs it; x is
host-cast to fp8e4m3 (the Q path tolerates it at ~2x error margin) which
keeps the weight/x ring light. Per 512-column chunk, one PSUM stream bank
runs K -> kT cast -> [Q -> q copies ->] V -> vT cast -> transposes -> v
copies; Q2/Q3 are deferred into the proj bank after its stream drains.
The exp stream on the scalar engine paces the attention phase; av matmuls
are emitted with a 4-slot lag so late v-copies cannot stall the score/exp
pipeline. bf16 ex/v/out datapath.

Self-contained: hardcodes all shapes; builds a Bass program and runs it via
concourse.bass_utils.run_bass_kernel_spmd on cores 0-7.
"""

import os
import sys

import numpy as np

# hardcoded problem shapes
H, E, D, MD, O, S = 16, 1024, 64, 64, 1024, 2048
NCORES = 8
HPC = H // NCORES          # heads per core = 2
DD = HPC * D               # packed head dim rows = 128
P = 128

# filled by the last device run (for test harness)
LAST_EXEC_TIME_NS = None
LAST_RESULTS = None

_REPO = "/opt/trn_rl_repo"
if _REPO not in sys.path:
    sys.path.insert(0, _REPO)

_built = {}


def _build_bass():
    import concourse.bass as bass
    import concourse.mybir as mybir

    F32 = mybir.dt.float32
    BF16 = mybir.dt.bfloat16
    F8 = mybir.dt.float8e4
    Exp = mybir.ActivationFunctionType.Exp

    nc = bass.Bass()
    import contextlib
    _lp = contextlib.ExitStack()
    _lp.enter_context(nc.allow_low_precision(
        reason="bf16/fp8 datapath is within the 2e-2 harness tolerance"))

    EC = E // P               # 8 e-chunks
    SC = S // 512             # 4 s/t-chunks of 512
    TB = S // P               # 16 t-blocks
    NG = SC * TB              # 64 score-blocks
    NEX = 12                  # exp sbuf slots
    NOB = 4                   # output staging slots of [P, 1024]
    LAG = 4                   # av emission lag (in g-slots) behind scores

    # prepacked inputs: chunk-major so every DMA is contiguous per partition
    xTp = nc.declare_dram_parameter("xTp", [SC, P, EC * 512], F8, isOutput=False)
    zTp = nc.declare_dram_parameter("zTp", [SC, P, EC * 512], BF16, isOutput=False)
    wq = nc.declare_dram_parameter("wq", [P, EC * DD], BF16, isOutput=False)
    wk = nc.declare_dram_parameter("wk", [P, EC * DD], BF16, isOutput=False)
    wv = nc.declare_dram_parameter("wv", [P, EC * DD], BF16, isOutput=False)
    bq = nc.declare_dram_parameter("bq", [DD, 1], F32, isOutput=False)
    w0 = nc.declare_dram_parameter("w0", [DD, O], BF16, isOutput=False)
    # partial outputs in bf16; the 8 partials are summed in fp32 on host
    out = nc.declare_dram_parameter("out", [S, O], BF16, isOutput=True)

    # ---- static SBUF allocation --------------------------------------
    xt_sb = nc.alloc_sbuf_tensor("xt_sb", [P, SC, EC, 512], F8).ap()
    zt_sb = nc.alloc_sbuf_tensor("zt_sb", [P, SC, EC, 512], BF16).ap()
    # padded q: cols 0:512 head0 (rows 64:128 zero), 512:1024 head1 (rows 0:64 zero)
    qP_sb = nc.alloc_sbuf_tensor("qP_sb", [P, SC, 1024], BF16).ap()
    kT_sb = nc.alloc_sbuf_tensor("kT_sb", [P, S], BF16).ap()
    wq_sb = nc.alloc_sbuf_tensor("wq_sb", [P, EC, DD], BF16).ap()
    wk_sb = nc.alloc_sbuf_tensor("wk_sb", [P, EC, DD], BF16).ap()
    wv_sb = nc.alloc_sbuf_tensor("wv_sb", [P, EC, DD], BF16).ap()
    w0_sb = nc.alloc_sbuf_tensor("w0_sb", [P, O], BF16).ap()
    bq_sb = nc.alloc_sbuf_tensor("bq_sb", [P, 1], F32).ap()
    ones_row = nc.alloc_sbuf_tensor("ones_row", [1, 64], BF16).ap()
    vT_sb = nc.alloc_sbuf_tensor("vT_sb", [P, S], BF16).ap()
    ident = nc.alloc_sbuf_tensor("ident", [P, P], BF16).ap()
    v0_sb = nc.alloc_sbuf_tensor("v0_sb", [P, TB, 65], BF16).ap()
    v1_sb = nc.alloc_sbuf_tensor("v1_sb", [P, TB, 65], BF16).ap()
    ex_sb = nc.alloc_sbuf_tensor("ex_sb", [P, NEX, 1024], BF16).ap()
    E_sb = nc.alloc_sbuf_tensor("E_sb", [P, 2, 512], F32).ap()
    rr_sb = nc.alloc_sbuf_tensor("rr_sb", [1, 2, 512], BF16).ap()
    lnt_sb = nc.alloc_sbuf_tensor("lnt_sb", [1, 2, 512], F32).ap()
    scr_sb = nc.alloc_sbuf_tensor("scr_sb", [1, 2], F32).ap()
    oT_sb = nc.alloc_sbuf_tensor("oT_sb", [P, 2, 512], BF16).ap()
    ob_sb = nc.alloc_sbuf_tensor("ob_sb", [P, NOB + 2, 1024], BF16).ap()

    # ---- static PSUM banks -------------------------------------------
    # qa0/qa1: scores ping-pong; kb[c]: per-chunk K/(Q/)V/tp stream bank,
    # later reused as av0/av1 (c=0,1) and bcast/proj (c=2,3).
    qa0 = nc.alloc_psum_tensor("qa0", [P, 1024], F32).ap()   # banks 0-1
    qa1 = nc.alloc_psum_tensor("qa1", [P, 1024], F32).ap()   # banks 2-3
    kb = [nc.alloc_psum_tensor(f"kb{c}", [P, 512], F32).ap()
          for c in range(SC)]                                # banks 4-7
    av0, av1, bcp, pjp = kb[0], kb[1], kb[2], kb[3]

    # q-proj psum bank per chunk: chunks 0/1 inside their stream bank,
    # chunks 2/3 through the proj bank after its stream fully drains
    QBANK = {0: kb[0], 1: kb[1], 2: kb[3], 3: kb[3]}

    def score_bank(g):
        return qa0 if g % 2 == 0 else qa1

    # ---- semaphores ---------------------------------------------------
    sQW = nc.alloc_semaphore("sQW")                          # wq+bq: 32
    sKW = nc.alloc_semaphore("sKW")                          # wk: 16
    sVW = nc.alloc_semaphore("sVW")                          # wv: 16
    sW0 = nc.alloc_semaphore("sW0")
    sXT = [nc.alloc_semaphore(f"sXT{c}") for c in range(SC)]
    sZT = [nc.alloc_semaphore(f"sZT{c}") for c in range(SC)]  # 4 pieces: 64
    sOB = [nc.alloc_semaphore(f"sOB{j}") for j in range(NOB)]
    sOBX = [nc.alloc_semaphore(f"sOBX{j}") for j in range(2)]
    sGP = nc.alloc_semaphore("sGP")
    sLN = nc.alloc_semaphore("sLN")
    sPE = nc.alloc_semaphore("sPE")
    sACT = nc.alloc_semaphore("sACT")
    sDVE = nc.alloc_semaphore("sDVE")

    # ---- PE order ----------------------------------------------------
    # scores lead (exp-paced), av lags LAG slots, chunk streams inserted
    # at the latest loop position that still meets their consumer deadline
    # given the modeled DMA arrival times.
    PE_ORD = [("k", 0), ("q", 0), ("sc", 0), ("sc", 1)]
    inserts = {
        0: [("v", 0), ("tp", 0), ("tp", 1), ("tp", 2), ("tp", 3)],
        1: [("k", 1), ("q", 1)],
        2: [("v", 1), ("tp", 4), ("tp", 5), ("tp", 6), ("tp", 7)],
        4: [("k", 2)],
        5: [("v", 2), ("tp", 8), ("tp", 9), ("tp", 10), ("tp", 11)],
        7: [("k", 3)],
        8: [("v", 3), ("tp", 12), ("tp", 13), ("tp", 14), ("tp", 15)],
        12: [("q", 2)],
        16: [("q", 3)],
    }
    defer_off = {6: [0], 8: [1], 9: [(0, 0), (0, 1)], 11: [(1, 0), (1, 1)],
                 13: [(2, 0), (2, 1)], 15: [(3, 0), (3, 1)]}
    for gl in range(NG + LAG):
        PE_ORD += inserts.get(gl, [])
        if gl + 2 < NG:
            PE_ORD.append(("sc", gl + 2))
        if 0 <= gl - LAG < NG:
            PE_ORD.append(("av", gl - LAG))
        # previous chunk's normalization-dependent PE work, deferred
        scq, off = divmod(gl, TB)
        if 1 <= scq <= SC - 1 and off in defer_off:
            pv = scq - 1
            for ent in defer_off[off]:
                if isinstance(ent, int):
                    PE_ORD.append(("bc", pv, ent))
                else:
                    PE_ORD.append(("pj", pv, ent[0], ent[1]))
    for h_ in range(2):
        PE_ORD.append(("bc", SC - 1, h_))
    for sb_ in range(4):
        for oc_ in range(2):
            PE_ORD.append(("pj", SC - 1, sb_, oc_))
    PE_TICK = {e: i + 1 for i, e in enumerate(PE_ORD)}

    def pe_k(c):
        return PE_TICK[("k", c)]

    def pe_q(c):
        return PE_TICK[("q", c)]

    def pe_v(c):
        return PE_TICK[("v", c)]

    def pe_tp(tb):
        return PE_TICK[("tp", tb)]

    def pe_scores(g):
        return PE_TICK[("sc", g)]

    def pe_av(g):
        return PE_TICK[("av", g)]

    def pe_bcast(sc, h):
        return PE_TICK[("bc", sc, h)]

    def pe_proj(sc, sb, oc):
        return PE_TICK[("pj", sc, sb, oc)]

    # ---- DVE ticks ----------------------------------------------------
    # chunks 0,1: kT, q-lo, q-hi, vT, vcopy x4 (8 each); chunks 2,3:
    # kT, vT, vcopy x4 (6 each); then q2/q3 copies (4); then per sc:
    # E-copy x2, recip x2, mult x2, ob x8 -> 14.
    def dve_kT(c):
        return 8 * c + 1 if c < 2 else 16 + 6 * (c - 2) + 1

    def dve_vT(c):
        return 8 * c + 4 if c < 2 else 16 + 6 * (c - 2) + 2

    def dve_vcopy(tb):
        c, j = divmod(tb, 4)
        return (8 * c + 5 + j) if c < 2 else (16 + 6 * (c - 2) + 3 + j)

    def dve_qlo(c):
        return 8 * c + 2 if c < 2 else 29 + 2 * (c - 2)

    def dve_qhi(c):
        return 8 * c + 3 if c < 2 else 30 + 2 * (c - 2)

    def dve_ecp(sc, h):
        return 32 + sc * 14 + h + 1

    def dve_rcp(sc, h):
        return 32 + sc * 14 + 2 + h + 1

    def dve_mult(sc, h):
        return 32 + sc * 14 + 4 + h + 1

    def dve_ob(gi):
        sc, j = divmod(gi, 8)
        return 32 + sc * 14 + 6 + j + 1

    # ACT: one tick per exp
    def act_exp(g):
        return g + 1

    def ob_slot(di):
        if di == 13:
            return NOB
        if di == 15:
            return NOB + 1
        return di % NOB

    # out-block DMA engine map: spreads the last chunk across all rings
    OUT_ENG = {di: ("sync" if di % 2 == 0 else "gp") for di in range(16)}
    OUT_ENG[13] = "gp"
    OUT_ENG[14] = "act"
    OUT_ENG[15] = "sync"

    counts = {"PE": 0, "ACT": 0, "DVE": 0}

    def inc(eng, instr, sem, expect):
        instr.then_inc(sem, 1)
        counts[eng] += 1
        assert counts[eng] == expect, (eng, counts[eng], expect)

    class WaitTracker:
        def __init__(self, eng):
            self.eng = eng
            self.seen = {}

        def need(self, sem, val):
            if val <= 0:
                return
            key = sem.name
            if self.seen.get(key, -1) >= val:
                return
            self.seen[key] = val
            self.eng.wait_ge(sem, val)

    def emit_out(eng, w, di):
        sc, sb = divmod(di, 4)
        row = sc * 512 + sb * P
        w.need(sDVE, dve_ob(sc * 8 + 2 * sb + 1))
        i = eng.dma_start(out=out[row:row + P, :], in_=ob_sb[:, ob_slot(di), :])
        if di == 13:
            i.then_inc(sOBX[0], 16)
        elif di == 15:
            i.then_inc(sOBX[1], 16)
        else:
            i.then_inc(sOB[di % NOB], 16)

    with nc.Block() as block:

        @block.sync
        def _(sp):
            # z quarter-pieces (2 e-chunks each), deadline-ordered
            for c, i in ((0, 0), (0, 1), (1, 0), (1, 1), (2, 0), (3, 0), (3, 1)):
                sp.dma_start(out=zt_sb[:, c, 2 * i:2 * i + 2, :],
                             in_=zTp[c][:, i * 1024:(i + 1) * 1024]
                             ).then_inc(sZT[c], 16)
            w = WaitTracker(sp)
            for di in range(16):
                if OUT_ENG[di] == "sync":
                    emit_out(sp, w, di)
            for j in range(NOB):
                nwrites = len([d for d in range(SC * 4) if d % NOB == j and ob_slot(d) == j])
                sp.wait_ge(sOB[j], 16 * nwrites)
            for j in range(2):
                sp.wait_ge(sOBX[j], 16)

        @block.gpsimd
        def _(gp):
            gp.dma_start(out=bq_sb, in_=bq[:, :]).then_inc(sQW, 16)
            for c, i in ((0, 2), (0, 3), (1, 2), (1, 3), (2, 1), (2, 2), (3, 2)):
                gp.dma_start(out=zt_sb[:, c, 2 * i:2 * i + 2, :],
                             in_=zTp[c][:, i * 1024:(i + 1) * 1024]
                             ).then_inc(sZT[c], 16)
            gp.wait_ge(sGP, 1)
            from concourse.masks import make_identity
            make_identity(nc, ident, nomemset=True)
            nc.gpsimd.engine_nop().then_inc(sGP, 1)
            gw = WaitTracker(gp)
            for di in range(16):
                if OUT_ENG[di] == "gp":
                    emit_out(gp, gw, di)

        @block.tensor
        def _(pe):
            w = WaitTracker(pe)
            for ent in PE_ORD:
                kind = ent[0]
                if kind == "k":
                    _, c = ent
                    w.need(sKW, 16)
                    w.need(sZT[c], 64)
                    for ec in range(EC):
                        i = nc.tensor.matmul(
                            kb[c][:, :],
                            lhsT=wk_sb[:, ec, :],
                            rhs=zt_sb[:, c, ec, :],
                            start=(ec == 0), stop=(ec == EC - 1),
                            skip_group_check=True,
                        )
                        if ec == EC - 1:
                            inc("PE", i, sPE, pe_k(c))
                elif kind == "q":
                    _, c = ent
                    w.need(sQW, 32)
                    w.need(sXT[c], 16)
                    if c < 2:
                        w.need(sDVE, dve_kT(c))
                    elif c == 2:
                        w.need(sDVE, dve_vcopy(15))
                    else:
                        w.need(sDVE, dve_qhi(2))
                    for ec in range(EC):
                        i = nc.tensor.matmul(
                            QBANK[c][:, :],
                            lhsT=wq_sb[:, ec, :],
                            rhs=xt_sb[:, c, ec, :],
                            start=(ec == 0), stop=(ec == EC - 1),
                            skip_group_check=True,
                        )
                        if ec == EC - 1:
                            inc("PE", i, sPE, pe_q(c))
                elif kind == "v":
                    _, c = ent
                    w.need(sVW, 16)
                    if c < 2:
                        w.need(sDVE, dve_qhi(c))
                    else:
                        w.need(sDVE, dve_kT(c))
                    for ec in range(EC):
                        i = nc.tensor.matmul(
                            kb[c][:, :],
                            lhsT=wv_sb[:, ec, :],
                            rhs=zt_sb[:, c, ec, :],
                            start=(ec == 0), stop=(ec == EC - 1),
                            skip_group_check=True,
                        )
                        if ec == EC - 1:
                            inc("PE", i, sPE, pe_v(c))
                elif kind == "tp":
                    _, tb = ent
                    c, j = divmod(tb, 4)
                    w.need(sGP, 2)
                    w.need(sDVE, dve_vT(c))
                    # kb[c] is shared by all 4 tp regions of the chunk: the
                    # previous vcopy (DVE read) must finish before this PE
                    # write to the same bank (bank collisions are fatal)
                    if j > 0:
                        w.need(sDVE, dve_vcopy(tb - 1))
                    tgt = kb[c][0:P, 64 * j:64 * j + 64].bitcast(BF16)
                    i = nc.tensor.transpose(
                        tgt, vT_sb[:, tb * P:(tb + 1) * P], ident)
                    inc("PE", i, sPE, pe_tp(tb))
                elif kind == "sc":
                    _, g = ent
                    sc, tb = divmod(g, TB)
                    qa = score_bank(g)
                    w.need(sDVE, dve_qhi(sc))
                    w.need(sDVE, dve_kT(tb // 4))
                    if g >= 2:
                        w.need(sACT, act_exp(g - 2))
                    # one shared kT stationary; two 512-wide streams (PSUM
                    # matmul output is limited to one bank)
                    nc.tensor.matmul(
                        qa[:, 0:512],
                        lhsT=kT_sb[:, tb * P:(tb + 1) * P],
                        rhs=qP_sb[:, sc, 0:512],
                        start=True, stop=True,
                    )
                    i = nc.tensor.matmul(
                        qa[:, 512:1024],
                        lhsT=kT_sb[:, tb * P:(tb + 1) * P],
                        rhs=qP_sb[:, sc, 512:1024],
                        start=True, stop=True,
                    )
                    inc("PE", i, sPE, pe_scores(g))
                elif kind == "av":
                    _, g = ent
                    sc, tb = divmod(g, TB)
                    w.need(sACT, act_exp(g))
                    w.need(sDVE, dve_vcopy(tb))
                    if g == 0:
                        w.need(sDVE, dve_vcopy(7))
                    if tb == 0 and sc > 0:
                        w.need(sDVE, dve_ecp(sc - 1, 1))
                    slot = g % NEX
                    nc.tensor.matmul(
                        av0[0:65, :],
                        lhsT=v0_sb[:, tb, :],
                        rhs=ex_sb[:, slot, 0:512],
                        start=(tb == 0), stop=(tb == TB - 1),
                        skip_group_check=True,
                    )
                    i = nc.tensor.matmul(
                        av1[0:65, :],
                        lhsT=v1_sb[:, tb, :],
                        rhs=ex_sb[:, slot, 512:1024],
                        start=(tb == 0), stop=(tb == TB - 1),
                        skip_group_check=True,
                    )
                    inc("PE", i, sPE, pe_av(g))
                elif kind == "bc":
                    _, sc, h = ent
                    if sc == SC - 1:
                        w.need(sACT, NG + h + 1)
                    else:
                        w.need(sDVE, dve_rcp(sc, h))
                    if h == 1:
                        w.need(sDVE, dve_mult(sc, 0))
                    elif sc > 0:
                        w.need(sDVE, dve_ob((sc - 1) * 8 + 7))
                    else:
                        w.need(sDVE, dve_vcopy(11))
                    i = nc.tensor.matmul(
                        bcp[0:64, :],
                        lhsT=ones_row[0:1, :],
                        rhs=rr_sb[0:1, h, :],
                        start=True, stop=True,
                    )
                    inc("PE", i, sPE, pe_bcast(sc, h))
                else:
                    _, sc, sb, oc = ent
                    gi = sc * 8 + sb * 2 + oc
                    j = sb * 2 + oc
                    if sc == SC - 1:
                        # scores are done: qa banks are free, so the tail
                        # projections fan out and do not wait on the ob ladder
                        bank = (qa0[:, 0:512], qa0[:, 512:1024],
                                qa1[:, 0:512], qa1[:, 512:1024],
                                pjp[:, :], bcp[:, :])[j % 6]
                        if j >= 6:
                            w.need(sDVE, dve_ob(sc * 8 + j - 6))
                    else:
                        bank = (pjp if gi % 2 == 0 else bcp)[:, :]
                        if gi >= 2:
                            w.need(sDVE, dve_ob(gi - 2))
                        if sc == 0:
                            w.need(sDVE, dve_qhi(3))
                    w.need(sW0, 16)
                    w.need(sDVE, dve_mult(sc, 1))
                    i = nc.tensor.matmul(
                        bank,
                        lhsT=oT_sb[:, sc % 2, sb * P:(sb + 1) * P],
                        rhs=w0_sb[:, oc * 512:(oc + 1) * 512],
                        start=True, stop=True,
                    )
                    inc("PE", i, sPE, pe_proj(sc, sb, oc))

        @block.scalar
        def _(act):
            w = WaitTracker(act)
            act.dma_start(out=wk_sb, in_=wk[:, :]).then_inc(sKW, 16)
            act.dma_start(out=wq_sb, in_=wq[:, :]).then_inc(sQW, 16)
            act.dma_start(out=xt_sb[:, 0, :, :], in_=xTp[0]).then_inc(sXT[0], 16)
            act.dma_start(out=wv_sb, in_=wv[:, :]).then_inc(sVW, 16)
            act.dma_start(out=xt_sb[:, 1, :, :], in_=xTp[1]).then_inc(sXT[1], 16)
            act.dma_start(out=zt_sb[:, 2, 6:8, :],
                          in_=zTp[2][:, 3 * 1024:4 * 1024]).then_inc(sZT[2], 16)
            act.dma_start(out=zt_sb[:, 3, 6:8, :],
                          in_=zTp[3][:, 3 * 1024:4 * 1024]).then_inc(sZT[3], 16)
            act.dma_start(out=xt_sb[:, 2, :, :], in_=xTp[2]).then_inc(sXT[2], 16)
            act.dma_start(out=xt_sb[:, 3, :, :], in_=xTp[3]).then_inc(sXT[3], 16)
            act.dma_start(out=w0_sb, in_=w0[:, :]).then_inc(sW0, 16)
            for g in range(NG):
                qa = score_bank(g)
                w.need(sPE, pe_scores(g))
                slot = g % NEX
                i = nc.scalar.activation(
                    ex_sb[:, slot, :], qa[:, :], Exp, scale=0.125)
                inc("ACT", i, sACT, act_exp(g))
            # last chunk's reciprocal via Ln+Exp (ACT is idle by then), so the
            # tail does not pay the ~4us DVE reciprocal
            for h in range(2):
                w.need(sDVE, dve_ecp(SC - 1, h))
                nc.scalar.activation(lnt_sb[0:1, h, :], E_sb[64:65, h, :],
                                     mybir.ActivationFunctionType.Ln).then_inc(sLN, 1)
                w.need(sLN, h + 1)
                i = nc.scalar.activation(rr_sb[0:1, h, :], lnt_sb[0:1, h, :],
                                         Exp, scale=-1.0)
                inc("ACT", i, sACT, NG + h + 1)
            for di in range(16):
                if OUT_ENG[di] == "act":
                    emit_out(act, w, di)

        @block.vector
        def _(dve):
            w = WaitTracker(dve)
            nc.vector.memset(ident, 0.0).then_inc(sGP, 1)
            nc.vector.memset(qP_sb[64:P, :, 0:512], 0.0)
            nc.vector.memset(qP_sb[0:64, :, 512:1024], 0.0)
            nc.vector.memset(ones_row, 1.0)
            nc.vector.memset(v0_sb[:, :, 64:65], 1.0)
            nc.vector.memset(v1_sb[:, :, 64:65], 1.0)

            def emit_q(c):
                w.need(sPE, pe_q(c))
                qa = QBANK[c][:, :]
                i = nc.vector.tensor_scalar_add(
                    out=qP_sb[0:64, c, 0:512],
                    in0=qa[0:64, :],
                    scalar1=bq_sb[0:64, 0:1],
                )
                inc("DVE", i, sDVE, dve_qlo(c))
                i = nc.vector.tensor_scalar_add(
                    out=qP_sb[64:P, c, 512:1024],
                    in0=qa[64:P, :],
                    scalar1=bq_sb[64:P, 0:1],
                )
                inc("DVE", i, sDVE, dve_qhi(c))

            for c in range(SC):
                # k: cast copy psum -> kT
                w.need(sPE, pe_k(c))
                i = nc.vector.tensor_copy(kT_sb[:, c * 512:(c + 1) * 512],
                                          kb[c][:, :])
                inc("DVE", i, sDVE, dve_kT(c))
                if c < 2:
                    emit_q(c)
                # vT: cast copy psum -> vT
                w.need(sPE, pe_v(c))
                i = nc.vector.tensor_copy(vT_sb[:, c * 512:(c + 1) * 512],
                                          kb[c][:, :])
                inc("DVE", i, sDVE, dve_vT(c))
                # v: split transposed [t, dd] blocks into per-head [t, 64]
                for j in range(4):
                    tb = 4 * c + j
                    src = kb[c][0:P, 64 * j:64 * j + 64].bitcast(BF16)
                    w.need(sPE, pe_tp(tb))
                    nc.vector.tensor_copy(v0_sb[:, tb, 0:64], src[:, 0:64])
                    i = nc.vector.tensor_copy(v1_sb[:, tb, 0:64], src[:, 64:128])
                    inc("DVE", i, sDVE, dve_vcopy(tb))
            emit_q(2)
            emit_q(3)
            # attention normalization + output staging
            for sc in range(SC):
                for h, av in ((0, av0), (1, av1)):
                    w.need(sPE, pe_av(sc * TB + TB - 1))
                    i = nc.vector.tensor_copy(E_sb[0:65, h, :], av[0:65, :])
                    inc("DVE", i, sDVE, dve_ecp(sc, h))
                for h in range(2):
                    w.need(sDVE, dve_ecp(sc, h))
                    if sc == SC - 1:
                        i = nc.vector.memset(scr_sb[0:1, h:h + 1], 0.0)
                    else:
                        i = nc.vector.reciprocal(rr_sb[0:1, h, :],
                                                 E_sb[64:65, h, :])
                    inc("DVE", i, sDVE, dve_rcp(sc, h))
                for h in range(2):
                    w.need(sPE, pe_bcast(sc, h))
                    i = nc.vector.tensor_mul(
                        oT_sb[h * 64:(h + 1) * 64, sc % 2, :],
                        E_sb[0:64, h, :], bcp[0:64, :])
                    inc("DVE", i, sDVE, dve_mult(sc, h))
                for j in range(8):
                    sb, oc = divmod(j, 2)
                    gi = sc * 8 + j
                    di = sc * 4 + sb
                    if sc == SC - 1:
                        bank = (qa0[:, 0:512], qa0[:, 512:1024],
                                qa1[:, 0:512], qa1[:, 512:1024],
                                pjp[:, :], bcp[:, :])[j % 6]
                    else:
                        bank = (pjp if gi % 2 == 0 else bcp)[:, :]
                    w.need(sPE, pe_proj(sc, sb, oc))
                    if di >= NOB and ob_slot(di) == di % NOB:
                        w.need(sOB[di % NOB], 16 * (di // NOB - (1 if di in (13 + NOB, 15 + NOB) else 0)))
                    i = nc.vector.tensor_copy(
                        ob_sb[:, ob_slot(di), oc * 512:(oc + 1) * 512], bank)
                    inc("DVE", i, sDVE, dve_ob(gi))

    _lp.close()
    return nc


def _get_nc():
    if "nc" not in _built:
        _built["nc"] = _build_bass()
    return _built["nc"]


def _pack_ts(aT):
    # [E, S] -> [SC, P, EC*512]: chunk-major, contiguous per partition;
    # row p*EC+ec of aT becomes partition p, segment ec (matches the
    # "(p c) d" rearrange of the SBUF layout)
    SCl = S // 512
    ECl = E // P
    return np.ascontiguousarray(
        aT.reshape(P, ECl, SCl, 512).transpose(2, 0, 1, 3).reshape(SCl, P, ECl * 512))


def _make_in_maps(x, z, Wq, bq, Wk, Wv, W0):
    import concourse.mybir as mybir
    BF = mybir.dt.np(mybir.dt.bfloat16)
    F8 = mybir.dt.np(mybir.dt.float8e4)
    xTp = _pack_ts(np.ascontiguousarray(x.T).astype(F8))
    zTp = _pack_ts(np.ascontiguousarray(z.T).astype(BF))
    ECl = E // P
    in_maps = []
    for c in range(NCORES):
        h0, h1 = 2 * c, 2 * c + 1
        wq_ = np.concatenate([Wq[h0], Wq[h1]], axis=1).astype(BF)
        wk_ = np.concatenate([Wk[h0], Wk[h1]], axis=1).astype(BF)
        wv_ = np.concatenate([Wv[h0], Wv[h1]], axis=1).astype(BF)
        pack_w = lambda a: np.ascontiguousarray(a.reshape(P, ECl * DD))
        in_maps.append({
            "xTp": xTp,
            "zTp": zTp,
            "wq": pack_w(wq_),
            "wk": pack_w(wk_),
            "wv": pack_w(wv_),
            "bq": np.ascontiguousarray(np.concatenate([bq[h0], bq[h1]]).reshape(DD, 1), np.float32),
            "w0": np.ascontiguousarray(W0[c * DD:(c + 1) * DD, :]).astype(BF),
        })
    return in_maps


def _numpy_reference(x, z, mask, Wq, bq, Wk, bk, Wv, bv, W0, b0):
    # general-mask fallback (not the benchmarked path; harness mask is all-ones)
    x = x.astype(np.float64); z = z.astype(np.float64)
    q = np.einsum("se,hed->hsd", x, Wq) + bq[:, None, :]
    k = np.einsum("te,hed->htd", z, Wk) + bk[:, None, :]
    v = np.einsum("te,hem->htm", z, Wv) + bv[:, None, :]
    s = np.einsum("hsd,htd->hst", q, k) / np.sqrt(np.float64(D))
    s = np.where(mask[None, :, :] == 0, -np.inf, s)
    s = s - s.max(axis=-1, keepdims=True)
    e = np.exp(s)
    a = e / e.sum(axis=-1, keepdims=True)
    o = np.einsum("hst,htm->hsm", a, v)
    o = np.transpose(o, (1, 0, 2)).reshape(S, H * MD)
    return (o @ W0 + b0).astype(np.float32)


def kernel(x, z, mask, Wq, bq, Wk, bk, Wv, bv, W0, b0):
    global LAST_EXEC_TIME_NS, LAST_RESULTS
    arrs = {k: np.asarray(v) for k, v in dict(
        x=x, z=z, mask=mask, Wq=Wq, bq=bq, Wk=Wk, bk=bk, Wv=Wv, bv=bv,
        W0=W0, b0=b0).items()}
    if not bool((arrs["mask"] != 0).all()):
        return _numpy_reference(**arrs)

    from concourse.bass_utils import run_bass_kernel_spmd

    nc = _get_nc()
    in_maps = _make_in_maps(
        arrs["x"], arrs["z"], arrs["Wq"], arrs["bq"], arrs["Wk"],
        arrs["Wv"], arrs["W0"])
    trace = bool(os.environ.get("KERNEL_TRACE"))
    kw = {}
    td = os.environ.get("KERNEL_TRACE_DIR")
    if td:
        os.makedirs(td, exist_ok=True)
        kw["tmpdir"] = td
    res = run_bass_kernel_spmd(
        nc, in_maps, core_ids=list(range(NCORES)), trace=trace, **kw
    )
    LAST_EXEC_TIME_NS = res.exec_time_ns
    LAST_RESULTS = res
    acc = np.zeros((S, O), dtype=np.float32)
    for rm in res.results:
        acc += rm["out"].astype(np.float32)
    # bv is not applied on-device: sum_t softmax * bv == bv, so it folds
    # into the final bias through W0.
    b0p = (arrs["b0"].astype(np.float64)
           + arrs["bv"].reshape(-1).astype(np.float64) @ arrs["W0"].astype(np.float64))
    acc += b0p.astype(np.float32)[None, :]
    return acc


# revision 32
# speedup vs baseline: 1.3324x; 1.3324x over previous
"""Multi-head attention (16 heads, S=2048, E=1024, D=M=64, O=1024) on 8 trn2
NeuronCores, head-sharded: 2 heads per core, partial output summed on host.

v2: bf16 matmul datapath (inputs host-cast), single-matmul scores via
zero-padded qT, direct [t,m] V projection (no transposes), fast reciprocal,
reordered DMA with split weight semaphores. bk is dropped (constant shift
along the softmax axis), bv is folded into b0 on host.

Self-contained: hardcodes all shapes; builds a Bass program and runs it via
concourse.bass_utils.run_bass_kernel_spmd on cores 0-7.
"""

import os
import sys

import numpy as np

# hardcoded problem shapes
H, E, D, MD, O, S = 16, 1024, 64, 64, 1024, 2048
NCORES = 8
HPC = H // NCORES          # heads per core = 2
DD = HPC * D               # packed head dim rows = 128
P = 128

# filled by the last device run (for test harness)
LAST_EXEC_TIME_NS = None
LAST_RESULTS = None

_REPO = "/opt/trn_rl_repo"
if _REPO not in sys.path:
    sys.path.insert(0, _REPO)

_built = {}


def _build_bass():
    import concourse.bass as bass
    import concourse.mybir as mybir

    F32 = mybir.dt.float32
    F32R = mybir.dt.float32r
    BF16 = mybir.dt.bfloat16
    Exp = mybir.ActivationFunctionType.Exp

    nc = bass.Bass()
    import contextlib
    _lp = contextlib.ExitStack()
    _lp.enter_context(nc.allow_low_precision(
        reason="bf16 datapath is within the 2e-2 harness tolerance"))

    EC2 = E // P
    xT = nc.declare_dram_parameter("xT", [E, S], BF16, isOutput=False)
    zT = nc.declare_dram_parameter("zT", [E, S], BF16, isOutput=False)
    # weights prepacked on host to the SBUF layout so the DMA is one
    # contiguous 2KB run per partition (vs 8x256B strided)
    wq = nc.declare_dram_parameter("wq", [P, EC2 * DD], BF16, isOutput=False)
    wk = nc.declare_dram_parameter("wk", [P, EC2 * DD], BF16, isOutput=False)
    wv = nc.declare_dram_parameter("wv", [P, EC2 * DD], BF16, isOutput=False)
    bq = nc.declare_dram_parameter("bq", [DD, 1], F32, isOutput=False)
    w0 = nc.declare_dram_parameter("w0", [DD, O], BF16, isOutput=False)
    # partial outputs in bf16: halves the output DMA + psum->sbuf staging;
    # the 8 partials are summed in fp32 on host
    out = nc.declare_dram_parameter("out", [S, O], BF16, isOutput=True)

    EC = E // P               # 8 e-chunks
    SC = S // 512             # 4 s-chunks of 512
    TB = S // P               # 16 t-blocks
    NEX = 12                  # exp sbuf slots
    NOB = 4                   # output staging slots of [P, 1024]

    # ---- static SBUF allocation --------------------------------------
    xt_sb = nc.alloc_sbuf_tensor("xt_sb", [P, EC, S], BF16).ap()
    zt_sb = nc.alloc_sbuf_tensor("zt_sb", [P, EC, S], BF16).ap()
    # padded q: cols 0:512 head0 (rows 64:128 zero), 512:1024 head1 (rows 0:64 zero)
    qP_sb = nc.alloc_sbuf_tensor("qP_sb", [P, SC, 1024], BF16).ap()
    kT_sb = nc.alloc_sbuf_tensor("kT_sb", [P, S], BF16).ap()
    wq_sb = nc.alloc_sbuf_tensor("wq_sb", [P, EC, DD], BF16).ap()
    wk_sb = nc.alloc_sbuf_tensor("wk_sb", [P, EC, DD], BF16).ap()
    wv_sb = nc.alloc_sbuf_tensor("wv_sb", [P, EC, DD], BF16).ap()
    w0_sb = nc.alloc_sbuf_tensor("w0_sb", [P, O], BF16).ap()
    bq_sb = nc.alloc_sbuf_tensor("bq_sb", [P, 1], F32).ap()
    ones_row = nc.alloc_sbuf_tensor("ones_row", [1, 64], BF16).ap()
    vT_sb = nc.alloc_sbuf_tensor("vT_sb", [P, S], BF16).ap()
    ident = nc.alloc_sbuf_tensor("ident", [P, P], BF16).ap()
    v0_sb = nc.alloc_sbuf_tensor("v0_sb", [P, TB, 65], BF16).ap()
    v1_sb = nc.alloc_sbuf_tensor("v1_sb", [P, TB, 65], BF16).ap()
    ex_sb = nc.alloc_sbuf_tensor("ex_sb", [P, NEX, 1024], BF16).ap()
    E_sb = nc.alloc_sbuf_tensor("E_sb", [P, 2, 512], F32).ap()
    dmy_sb = nc.alloc_sbuf_tensor("dmy_sb", [P, 512], BF16).ap()
    rr_sb = nc.alloc_sbuf_tensor("rr_sb", [1, 2, 512], BF16).ap()
    lnt_sb = nc.alloc_sbuf_tensor("lnt_sb", [1, 2, 512], F32).ap()
    scr_sb = nc.alloc_sbuf_tensor("scr_sb", [1, 2], F32).ap()
    oT_sb = nc.alloc_sbuf_tensor("oT_sb", [P, 2, 512], BF16).ap()
    ob_sb = nc.alloc_sbuf_tensor("ob_sb", [P, NOB + 2, 1024], BF16).ap()

    # ---- static PSUM banks -------------------------------------------
    qa0 = nc.alloc_psum_tensor("qa0", [P, 1024], F32).ap()   # banks 0-1
    qa1 = nc.alloc_psum_tensor("qa1", [P, 1024], F32).ap()   # banks 2-3
    av0 = nc.alloc_psum_tensor("av0", [P, 512], F32).ap()    # bank 4
    av1 = nc.alloc_psum_tensor("av1", [P, 512], F32).ap()    # bank 5
    bcp = nc.alloc_psum_tensor("bcp", [P, 512], F32).ap()    # bank 6
    pjp = nc.alloc_psum_tensor("pjp", [P, 512], F32).ap()    # bank 7

    # ---- semaphores ---------------------------------------------------
    sQW = nc.alloc_semaphore("sQW")                          # wq+bq: 32
    sKW = nc.alloc_semaphore("sKW")                          # wk: 16
    sVW = nc.alloc_semaphore("sVW")                          # wv: 16
    sW0 = nc.alloc_semaphore("sW0")
    sXT = [nc.alloc_semaphore(f"sXT{c}") for c in range(4)]
    sZT = [nc.alloc_semaphore(f"sZT{c}") for c in range(4)]
    sOB = [nc.alloc_semaphore(f"sOB{j}") for j in range(NOB)]
    sOBX = [nc.alloc_semaphore(f"sOBX{j}") for j in range(2)]
    sGP = nc.alloc_semaphore("sGP")
    sLN = nc.alloc_semaphore("sLN")
    sWU = nc.alloc_semaphore("sWU")
    sQP = nc.alloc_semaphore("sQP")
    sPE = nc.alloc_semaphore("sPE")
    sACT = nc.alloc_semaphore("sACT")
    sDVE = nc.alloc_semaphore("sDVE")

    # ---- closed-form tick schedules ----------------------------------
    # PE ticks: q sc (4), k sc (4), v tb (16), then attention entries.
    def pe_q(sc):
        return sc + 1

    def pe_k(sc):
        return 4 + sc + 1

    def pe_vT(sc):
        return 8 + sc + 1

    def pe_tp(tb):
        return 12 + tb + 1

    def pe_scores(sc, tb):
        return PE_TICK[("scores", sc, tb)]

    def pe_av(sc, tb):
        return PE_TICK[("av", sc, tb)]

    def pe_bcast(sc, h):
        return PE_TICK[("bcast", sc, h)]

    def pe_proj(sc, sb, oc):
        return PE_TICK[("proj", sc, sb, oc)]

    # ACT: one tick per exp
    def act_exp(sc, tb):
        return sc * TB + tb + 1

    # DVE ticks: q sc (4), k sc (4), v tb (16), then per sc:
    # E-copy x2, recip x2, mult x2, ob x8 -> 14 per sc.
    def dve_q(sc):
        return sc + 1

    def dve_k(sc):
        return 4 + sc + 1

    def dve_vT(sc):
        return 8 + sc + 1

    def dve_vcopy(tb):
        return 12 + tb + 1

    def dve_ecp(sc, h):
        return 28 + sc * 14 + h + 1

    def dve_rcp(sc, h):
        return 28 + sc * 14 + 2 + h + 1

    def dve_mult(sc, h):
        return 28 + sc * 14 + 4 + h + 1

    def dve_ob(gi):
        sc, j = divmod(gi, 8)
        return 28 + sc * 14 + 6 + j + 1

    # software-pipelined PE attention order: scores run 2 iterations
    # ahead of AV.
    ATTN_ORD = [("scores", 0, 0), ("scores", 0, 1)]
    for sc_ in range(SC):
        for tb_ in range(TB):
            gn = sc_ * TB + tb_ + 2
            if gn < SC * TB:
                ATTN_ORD.append(("scores", gn // TB, gn % TB))
            ATTN_ORD.append(("av", sc_, tb_))
            # previous chunk's normalization-dependent PE work, deferred
            # deep enough that the ~4us reciprocals run off the critical path
            if sc_ > 0:
                pv = sc_ - 1
                for j_ in {6: [-1], 9: [-2], 10: [0], 11: [1, 2], 12: [3, 4],
                           13: [5, 6], 14: [7]}.get(tb_, []):
                    if j_ == -1:
                        ATTN_ORD.append(("bcast", pv, 0))
                    elif j_ == -2:
                        ATTN_ORD.append(("bcast", pv, 1))
                    else:
                        ATTN_ORD.append(("proj", pv, j_ // 2, j_ % 2))
    for h_ in range(2):
        ATTN_ORD.append(("bcast", SC - 1, h_))
    for sb_ in range(4):
        for oc_ in range(2):
            ATTN_ORD.append(("proj", SC - 1, sb_, oc_))
    PE_TICK = {e: 28 + i + 1 for i, e in enumerate(ATTN_ORD)}

    def ob_slot(di):
        if di == 13:
            return NOB
        if di == 15:
            return NOB + 1
        return di % NOB

    counts = {"PE": 0, "ACT": 0, "DVE": 0}

    def inc(eng, instr, sem, expect):
        instr.then_inc(sem, 1)
        counts[eng] += 1
        assert counts[eng] == expect, (eng, counts[eng], expect)

    class WaitTracker:
        def __init__(self, eng):
            self.eng = eng
            self.seen = {}

        def need(self, sem, val):
            if val <= 0:
                return
            key = sem.name
            if self.seen.get(key, -1) >= val:
                return
            self.seen[key] = val
            self.eng.wait_ge(sem, val)

    with nc.Block() as block:

        @block.sync
        def _(sp):
            sp.dma_start(out=wq_sb, in_=wq[:, :]).then_inc(sQW, 16)
            sp.dma_start(out=bq_sb, in_=bq[:, :]).then_inc(sQW, 16)
            xr = xT.rearrange("(p c) d -> p c d", p=P)
            for qi in range(4):
                sp.dma_start(out=xt_sb[:, 2 * qi:2 * qi + 2, :],
                             in_=xr[:, 2 * qi:2 * qi + 2, :]).then_inc(sXT[qi], 16)
            sp.dma_start(out=w0_sb, in_=w0[:, :]).then_inc(sW0, 16)
            w = WaitTracker(sp)
            for sc in range(SC):
                for sb in range(4):
                    di = sc * 4 + sb
                    if di % 2 != 0 or di == 14:
                        continue
                    row = sc * 512 + sb * P
                    w.need(sDVE, dve_ob(sc * 8 + 2 * sb + 1))
                    sp.dma_start(
                        out=out[row:row + P, :],
                        in_=ob_sb[:, ob_slot(di), :],
                    ).then_inc(sOB[di % NOB], 16)
            for j in range(NOB):
                nwrites = len([d for d in range(SC * 4) if d % NOB == j and ob_slot(d) == j])
                sp.wait_ge(sOB[j], 16 * nwrites)
            for j in range(2):
                sp.wait_ge(sOBX[j], 16)
            if os.environ.get("KDBG"):
                sDBG = nc.alloc_semaphore("sDBG")
                d_qP = nc.declare_dram_parameter("d_qP", [P, SC * 1024], mybir.dt.bfloat16, isOutput=True)
                d_kT = nc.declare_dram_parameter("d_kT", [P, S], mybir.dt.bfloat16, isOutput=True)
                d_v0 = nc.declare_dram_parameter("d_v0", [P, TB * 65], mybir.dt.bfloat16, isOutput=True)
                d_v1 = nc.declare_dram_parameter("d_v1", [P, TB * 65], mybir.dt.bfloat16, isOutput=True)
                d_ex = nc.declare_dram_parameter("d_ex", [P, NEX * 1024], mybir.dt.bfloat16, isOutput=True)
                d_rr = nc.declare_dram_parameter("d_rr", [1, 2 * 512], mybir.dt.bfloat16, isOutput=True)
                d_oT = nc.declare_dram_parameter("d_oT", [P, 2 * 512], mybir.dt.bfloat16, isOutput=True)
                sp.dma_start(out=d_qP[:, :], in_=qP_sb).then_inc(sDBG, 16)
                sp.dma_start(out=d_kT[:, :], in_=kT_sb).then_inc(sDBG, 16)
                sp.dma_start(out=d_v0[:, :], in_=v0_sb).then_inc(sDBG, 16)
                sp.dma_start(out=d_v1[:, :], in_=v1_sb).then_inc(sDBG, 16)
                sp.dma_start(out=d_ex[:, :], in_=ex_sb).then_inc(sDBG, 16)
                sp.dma_start(out=d_rr[:, :], in_=rr_sb).then_inc(sDBG, 16)
                sp.dma_start(out=d_oT[:, :], in_=oT_sb).then_inc(sDBG, 16)
                sp.wait_ge(sDBG, 16 * 7)

        @block.gpsimd
        def _(gp):
            gp.wait_ge(sGP, 1)
            from concourse.masks import make_identity
            make_identity(nc, ident, nomemset=True)
            nc.gpsimd.engine_nop().then_inc(sGP, 1)
            gw = WaitTracker(gp)
            for sc in range(SC):
                for sb in range(4):
                    di = sc * 4 + sb
                    if (di % 2 != 1 and di != 14) or di in (13, 15):
                        continue
                    row = sc * 512 + sb * P
                    gw.need(sDVE, dve_ob(sc * 8 + 2 * sb + 1))
                    gp.dma_start(
                        out=out[row:row + P, :],
                        in_=ob_sb[:, ob_slot(di), :],
                    ).then_inc(sOB[di % NOB], 16)

        @block.tensor
        def _(pe):
            w = WaitTracker(pe)
            kbank = (av0, av1, bcp, pjp)
            # HAM warm-up: dummy matmuls keep the PE busy from t~0 so the
            # p-state reaches full clock before the real projections start.
            w.need(sWU, 1)
            for _wu in range(20):
                nc.tensor.matmul(
                    av0[:, :],
                    lhsT=dmy_sb[:, 0:128],
                    rhs=dmy_sb[:, :],
                    start=True, stop=True,
                    skip_group_check=True,
                )
            # Q (qa banks) and K (banks 4-7) interleaved per input quarter.
            for qi in range(4):
                w.need(sQW, 32)
                w.need(sXT[qi], 16)
                for e in range(2):
                    ec = 2 * qi + e
                    for sc in range(SC):
                        i = nc.tensor.matmul(
                            (qa0 if sc < 2 else qa1)[:, (sc % 2) * 512:(sc % 2) * 512 + 512],
                            lhsT=wq_sb[:, ec, :],
                            rhs=xt_sb[:, ec, sc * 512:(sc + 1) * 512],
                            start=(ec == 0), stop=(ec == EC - 1),
                            skip_group_check=True,
                        )
                        if ec == EC - 1:
                            inc("PE", i, sPE, pe_q(sc))
                w.need(sKW, 16)
                w.need(sZT[qi], 16)
                for e in range(2):
                    ec = 2 * qi + e
                    for sc in range(SC):
                        i = nc.tensor.matmul(
                            kbank[sc][:, :],
                            lhsT=wk_sb[:, ec, :],
                            rhs=zt_sb[:, ec, sc * 512:(sc + 1) * 512],
                            start=(ec == 0), stop=(ec == EC - 1),
                            skip_group_check=True,
                        )
                        if ec == EC - 1:
                            inc("PE", i, sPE, pe_k(sc))
            # V projection as vT [dd, t] into qa banks (after q drains),
            # chunk-at-a-time so each starts as soon as its q copy lands.
            for sc in range(SC):
                w.need(sVW, 16)
                w.need(sDVE, dve_q(sc))
                w.need(sQP, sc + 1)
                for ec in range(EC):
                    i = nc.tensor.matmul(
                        (qa0 if sc < 2 else qa1)[:, (sc % 2) * 512:(sc % 2) * 512 + 512],
                        lhsT=wv_sb[:, ec, :],
                        rhs=zt_sb[:, ec, sc * 512:(sc + 1) * 512],
                        start=(ec == 0), stop=(ec == EC - 1),
                        skip_group_check=True,
                    )
                    if ec == EC - 1:
                        inc("PE", i, sPE, pe_vT(sc))
            w.need(sGP, 2)
            for tb in range(TB):
                tgt = (bcp if tb % 2 == 0 else pjp)[0:P, 0:64].bitcast(BF16)
                w.need(sDVE, dve_vT(tb // 4))
                w.need(sDVE, dve_k(2 if tb % 2 == 0 else 3))
                if tb >= 2:
                    w.need(sDVE, dve_vcopy(tb - 2))
                i = nc.tensor.transpose(tgt, vT_sb[:, tb * P:(tb + 1) * P], ident)
                inc("PE", i, sPE, pe_tp(tb))
            # Attention + projection, software-pipelined.
            for ent in ATTN_ORD:
                kind = ent[0]
                if kind == "scores":
                    _, sc, tb = ent
                    g = sc * TB + tb
                    qa = qa0 if tb % 2 == 0 else qa1
                    w.need(sDVE, dve_q(sc))
                    w.need(sQP, sc + 1)
                    w.need(sDVE, dve_k(tb // 4))
                    # qa bank pair was last read by the vT copies of the
                    # two projection chunks it held
                    w.need(sDVE, dve_vT(1 if tb % 2 == 0 else 3))
                    if g >= 2:
                        w.need(sACT, g - 1)
                    # one shared kT stationary; two 512-wide streams (PSUM
                    # matmul output is limited to one bank)
                    nc.tensor.matmul(
                        qa[:, 0:512],
                        lhsT=kT_sb[:, tb * P:(tb + 1) * P],
                        rhs=qP_sb[:, sc, 0:512],
                        start=True, stop=True,
                    )
                    i = nc.tensor.matmul(
                        qa[:, 512:1024],
                        lhsT=kT_sb[:, tb * P:(tb + 1) * P],
                        rhs=qP_sb[:, sc, 512:1024],
                        start=True, stop=True,
                    )
                    inc("PE", i, sPE, pe_scores(sc, tb))
                elif kind == "av":
                    _, sc, tb = ent
                    g = sc * TB + tb
                    if tb == 0 and sc > 0:
                        w.need(sDVE, dve_ecp(sc - 1, 1))
                    if tb == 0 and sc == 0:
                        w.need(sDVE, dve_k(1))
                    w.need(sDVE, dve_vcopy(tb))
                    w.need(sACT, act_exp(sc, tb))
                    slot = g % NEX
                    nc.tensor.matmul(
                        av0[0:65, :],
                        lhsT=v0_sb[:, tb, :],
                        rhs=ex_sb[:, slot, 0:512],
                        start=(tb == 0), stop=(tb == TB - 1),
                        skip_group_check=True,
                    )
                    i = nc.tensor.matmul(
                        av1[0:65, :],
                        lhsT=v1_sb[:, tb, :],
                        rhs=ex_sb[:, slot, 512:1024],
                        start=(tb == 0), stop=(tb == TB - 1),
                        skip_group_check=True,
                    )
                    inc("PE", i, sPE, pe_av(sc, tb))
                elif kind == "bcast":
                    _, sc, h = ent
                    if sc == SC - 1:
                        w.need(sACT, SC * TB + h + 1)
                    else:
                        w.need(sDVE, dve_rcp(sc, h))
                    if h == 1:
                        w.need(sDVE, dve_mult(sc, 0))
                    elif sc > 0:
                        w.need(sDVE, dve_ob((sc - 1) * 8 + 7))
                    i = nc.tensor.matmul(
                        bcp[0:64, :],
                        lhsT=ones_row[0:1, :],
                        rhs=rr_sb[0:1, h, :],
                        start=True, stop=True,
                    )
                    inc("PE", i, sPE, pe_bcast(sc, h))
                else:
                    _, sc, sb, oc = ent
                    gi = sc * 8 + sb * 2 + oc
                    j = sb * 2 + oc
                    if sc == SC - 1:
                        # scores are done: qa banks are free, so the tail
                        # projections fan out and do not wait on the ob ladder
                        bank = (qa0[:, 0:512], qa0[:, 512:1024],
                                qa1[:, 0:512], qa1[:, 512:1024],
                                pjp[:, :], bcp[:, :])[j % 6]
                        if j >= 6:
                            w.need(sDVE, dve_ob(sc * 8 + j - 6))
                    else:
                        bank = (pjp if gi % 2 == 0 else bcp)[:, :]
                        if gi >= 2:
                            w.need(sDVE, dve_ob(gi - 2))
                    w.need(sW0, 16)
                    w.need(sDVE, dve_mult(sc, 1))
                    i = nc.tensor.matmul(
                        bank,
                        lhsT=oT_sb[:, sc % 2, sb * P:(sb + 1) * P],
                        rhs=w0_sb[:, oc * 512:(oc + 1) * 512],
                        start=True, stop=True,
                    )
                    inc("PE", i, sPE, pe_proj(sc, sb, oc))

        @block.scalar
        def _(act):
            w = WaitTracker(act)
            zr = zT.rearrange("(p c) d -> p c d", p=P)
            act.dma_start(out=wk_sb, in_=wk[:, :]).then_inc(sKW, 16)
            act.dma_start(out=wv_sb, in_=wv[:, :]).then_inc(sVW, 16)
            for qi in range(4):
                act.dma_start(out=zt_sb[:, 2 * qi:2 * qi + 2, :],
                              in_=zr[:, 2 * qi:2 * qi + 2, :]).then_inc(sZT[qi], 16)
            Ident = mybir.ActivationFunctionType.Identity
            for sc in range(SC):
                w.need(sQW, 32)
                w.need(sPE, pe_q(sc))
                qa = (qa0 if sc < 2 else qa1)[:, (sc % 2) * 512:(sc % 2) * 512 + 512]
                nc.scalar.activation(
                    qP_sb[64:P, sc, 512:1024], qa[64:P, :], Ident,
                    bias=bq_sb[64:P, 0:1]).then_inc(sQP, 1)
            for sc in range(SC):
                for tb in range(TB):
                    g = sc * TB + tb
                    w.need(sPE, pe_scores(sc, tb))
                    if g >= NEX:
                        # slot-reuse wait on pe_av(g-NEX) is subsumed by the
                        # pe_scores wait (asserted below at build time)
                        gp_sc, gp_tb = divmod(g - NEX, TB)
                        assert pe_av(gp_sc, gp_tb) < pe_scores(sc, tb)
                    slot = g % NEX
                    qa = qa0 if tb % 2 == 0 else qa1
                    i = nc.scalar.activation(
                        ex_sb[:, slot, :], qa[:, :], Exp, scale=0.125)
                    inc("ACT", i, sACT, act_exp(sc, tb))
            # last chunk's reciprocal via Ln+Exp (ACT is idle by then), so the
            # tail does not pay the ~4us DVE reciprocal
            for h in range(2):
                w.need(sDVE, dve_ecp(SC - 1, h))
                nc.scalar.activation(lnt_sb[0:1, h, :], E_sb[64:65, h, :],
                                     mybir.ActivationFunctionType.Ln).then_inc(sLN, 1)
                w.need(sLN, h + 1)
                i = nc.scalar.activation(rr_sb[0:1, h, :], lnt_sb[0:1, h, :],
                                         Exp, scale=-1.0)
                inc("ACT", i, sACT, SC * TB + h + 1)
            for xi, di in enumerate((13, 15)):
                sb = di % 4
                row = (SC - 1) * 512 + sb * P
                w.need(sDVE, dve_ob((SC - 1) * 8 + 2 * sb + 1))
                act.dma_start(
                    out=out[row:row + P, :],
                    in_=ob_sb[:, ob_slot(di), :],
                ).then_inc(sOBX[xi], 16)

        @block.vector
        def _(dve):
            w = WaitTracker(dve)
            # zero the q pads once; later ticks imply completion (in-order)
            nc.vector.memset(dmy_sb, 0.0).then_inc(sWU, 1)
            nc.vector.memset(ident, 0.0).then_inc(sGP, 1)
            nc.vector.memset(qP_sb[64:P, :, 0:512], 0.0)
            nc.vector.memset(qP_sb[0:64, :, 512:1024], 0.0)
            nc.vector.memset(ones_row, 1.0)
            nc.vector.memset(v0_sb[:, :, 64:65], 1.0)
            nc.vector.memset(v1_sb[:, :, 64:65], 1.0)
            # q: bias-add + cast into padded layout (head0 half; head1 on ACT)
            for sc in range(SC):
                w.need(sQW, 32)
                w.need(sPE, pe_q(sc))
                qa = (qa0 if sc < 2 else qa1)[:, (sc % 2) * 512:(sc % 2) * 512 + 512]
                i = nc.vector.tensor_scalar_add(
                    out=qP_sb[0:64, sc, 0:512],
                    in0=qa[0:64, :],
                    scalar1=bq_sb[0:64, 0:1],
                )
                inc("DVE", i, sDVE, dve_q(sc))
            # k: plain cast copy out of banks 4-7
            kbank = (av0, av1, bcp, pjp)
            for sc in range(SC):
                w.need(sPE, pe_k(sc))
                i = nc.vector.tensor_copy(kT_sb[:, sc * 512:(sc + 1) * 512],
                                          kbank[sc][:, :])
                inc("DVE", i, sDVE, dve_k(sc))
            # vT: cast copy out of qa banks
            for sc in range(SC):
                w.need(sPE, pe_vT(sc))
                qa = (qa0 if sc < 2 else qa1)[:, (sc % 2) * 512:(sc % 2) * 512 + 512]
                i = nc.vector.tensor_copy(vT_sb[:, sc * 512:(sc + 1) * 512], qa)
                inc("DVE", i, sDVE, dve_vT(sc))
            # v: split transposed [t, dd] blocks into per-head [t, 64] slots
            for tb in range(TB):
                src = (bcp if tb % 2 == 0 else pjp)[0:P, 0:64].bitcast(BF16)
                w.need(sPE, pe_tp(tb))
                nc.vector.tensor_copy(v0_sb[:, tb, 0:64], src[:, 0:64])
                i = nc.vector.tensor_copy(v1_sb[:, tb, 0:64], src[:, 64:128])
                inc("DVE", i, sDVE, dve_vcopy(tb))
            # attention normalization + output staging
            for sc in range(SC):
                for h, av in ((0, av0), (1, av1)):
                    w.need(sPE, pe_av(sc, TB - 1))
                    i = nc.vector.tensor_copy(E_sb[0:65, h, :], av[0:65, :])
                    inc("DVE", i, sDVE, dve_ecp(sc, h))
                for h in range(2):
                    w.need(sDVE, dve_ecp(sc, h))
                    if sc == SC - 1:
                        i = nc.vector.memset(scr_sb[0:1, h:h + 1], 0.0)
                    else:
                        i = nc.vector.reciprocal(rr_sb[0:1, h, :],
                                                 E_sb[64:65, h, :])
                    inc("DVE", i, sDVE, dve_rcp(sc, h))
                for h in range(2):
                    w.need(sPE, pe_bcast(sc, h))
                    i = nc.vector.tensor_mul(
                        oT_sb[h * 64:(h + 1) * 64, sc % 2, :],
                        E_sb[0:64, h, :], bcp[0:64, :])
                    inc("DVE", i, sDVE, dve_mult(sc, h))
                for j in range(8):
                    sb, oc = divmod(j, 2)
                    gi = sc * 8 + j
                    di = sc * 4 + sb
                    if sc == SC - 1:
                        bank = (qa0[:, 0:512], qa0[:, 512:1024],
                                qa1[:, 0:512], qa1[:, 512:1024],
                                pjp[:, :], bcp[:, :])[j % 6]
                    else:
                        bank = (pjp if gi % 2 == 0 else bcp)[:, :]
                    w.need(sPE, pe_proj(sc, sb, oc))
                    if di >= NOB and ob_slot(di) == di % NOB:
                        w.need(sOB[di % NOB], 16 * (di // NOB - (1 if di in (13 + NOB, 15 + NOB) else 0)))
                    i = nc.vector.tensor_copy(
                        ob_sb[:, ob_slot(di), oc * 512:(oc + 1) * 512], bank)
                    inc("DVE", i, sDVE, dve_ob(gi))

    _lp.close()
    return nc


def _get_nc():
    if "nc" not in _built:
        _built["nc"] = _build_bass()
    return _built["nc"]


def _make_in_maps(x, z, Wq, bq, Wk, Wv, W0):
    import concourse.mybir as mybir
    BF = mybir.dt.np(mybir.dt.bfloat16)
    xT = np.ascontiguousarray(x.T).astype(BF)
    zT = np.ascontiguousarray(z.T).astype(BF)
    in_maps = []
    for c in range(NCORES):
        h0, h1 = 2 * c, 2 * c + 1
        pack_w = lambda a: np.ascontiguousarray(
            np.concatenate(a, axis=1).astype(BF).reshape(P, (E // P) * DD))
        in_maps.append({
            "xT": xT,
            "zT": zT,
            "wq": pack_w([Wq[h0], Wq[h1]]),
            "wk": pack_w([Wk[h0], Wk[h1]]),
            "wv": pack_w([Wv[h0], Wv[h1]]),
            "bq": np.ascontiguousarray(np.concatenate([bq[h0], bq[h1]]).reshape(DD, 1), np.float32),
            "w0": np.ascontiguousarray(W0[c * DD:(c + 1) * DD, :]).astype(BF),
        })
    return in_maps


def _numpy_reference(x, z, mask, Wq, bq, Wk, bk, Wv, bv, W0, b0):
    # general-mask fallback (not the benchmarked path; harness mask is all-ones)
    x = x.astype(np.float64); z = z.astype(np.float64)
    q = np.einsum("se,hed->hsd", x, Wq) + bq[:, None, :]
    k = np.einsum("te,hed->htd", z, Wk) + bk[:, None, :]
    v = np.einsum("te,hem->htm", z, Wv) + bv[:, None, :]
    s = np.einsum("hsd,htd->hst", q, k) / np.sqrt(np.float64(D))
    s = np.where(mask[None, :, :] == 0, -np.inf, s)
    s = s - s.max(axis=-1, keepdims=True)
    e = np.exp(s)
    a = e / e.sum(axis=-1, keepdims=True)
    o = np.einsum("hst,htm->hsm", a, v)
    o = np.transpose(o, (1, 0, 2)).reshape(S, H * MD)
    return (o @ W0 + b0).astype(np.float32)


def kernel(x, z, mask, Wq, bq, Wk, bk, Wv, bv, W0, b0):
    global LAST_EXEC_TIME_NS, LAST_RESULTS
    arrs = {k: np.asarray(v) for k, v in dict(
        x=x, z=z, mask=mask, Wq=Wq, bq=bq, Wk=Wk, bk=bk, Wv=Wv, bv=bv,
        W0=W0, b0=b0).items()}
    if not bool((arrs["mask"] != 0).all()):
        return _numpy_reference(**arrs)

    from concourse.bass_utils import run_bass_kernel_spmd

    nc = _get_nc()
    in_maps = _make_in_maps(
        arrs["x"], arrs["z"], arrs["Wq"], arrs["bq"], arrs["Wk"],
        arrs["Wv"], arrs["W0"])
    trace = bool(os.environ.get("KERNEL_TRACE"))
    kw = {}
    td = os.environ.get("KERNEL_TRACE_DIR")
    if td:
        os.makedirs(td, exist_ok=True)
        kw["tmpdir"] = td
    res = run_bass_kernel_spmd(
        nc, in_maps, core_ids=list(range(NCORES)), trace=trace, **kw
    )
    LAST_EXEC_TIME_NS = res.exec_time_ns
    LAST_RESULTS = res
    acc = np.zeros((S, O), dtype=np.float32)
    for rm in res.results:
        acc += rm["out"].astype(np.float32)
    # bv is not applied on-device: sum_t softmax * bv == bv, so it folds
    # into the final bias through W0.
    b0p = (arrs["b0"].astype(np.float64)
           + arrs["bv"].reshape(-1).astype(np.float64) @ arrs["W0"].astype(np.float64))
    acc += b0p.astype(np.float32)[None, :]
    return acc



# revision 33
# speedup vs baseline: 1.3514x; 1.0142x over previous
"""Multi-head attention (16 heads, S=2048, E=1024, D=M=64, O=1024) on 8 trn2
NeuronCores, head-sharded: 2 heads per core, partial output summed on host.

v2: bf16 matmul datapath (inputs host-cast), single-matmul scores via
zero-padded qT, direct [t,m] V projection (no transposes), fast reciprocal,
reordered DMA with split weight semaphores. bk is dropped (constant shift
along the softmax axis), bv is folded into b0 on host.

Self-contained: hardcodes all shapes; builds a Bass program and runs it via
concourse.bass_utils.run_bass_kernel_spmd on cores 0-7.
"""

import os
import sys

import numpy as np

# hardcoded problem shapes
H, E, D, MD, O, S = 16, 1024, 64, 64, 1024, 2048
NCORES = 8
HPC = H // NCORES          # heads per core = 2
DD = HPC * D               # packed head dim rows = 128
P = 128

# filled by the last device run (for test harness)
LAST_EXEC_TIME_NS = None
LAST_RESULTS = None

_REPO = "/opt/trn_rl_repo"
if _REPO not in sys.path:
    sys.path.insert(0, _REPO)

_built = {}


def _build_bass():
    import concourse.bass as bass
    import concourse.mybir as mybir

    F32 = mybir.dt.float32
    F32R = mybir.dt.float32r
    BF16 = mybir.dt.bfloat16
    Exp = mybir.ActivationFunctionType.Exp

    nc = bass.Bass()
    import contextlib
    _lp = contextlib.ExitStack()
    _lp.enter_context(nc.allow_low_precision(
        reason="bf16 datapath is within the 2e-2 harness tolerance"))

    xT = nc.declare_dram_parameter("xT", [E, S], BF16, isOutput=False)
    zT = nc.declare_dram_parameter("zT", [E, S], BF16, isOutput=False)
    wq = nc.declare_dram_parameter("wq", [E, DD], BF16, isOutput=False)
    wk = nc.declare_dram_parameter("wk", [E, DD], BF16, isOutput=False)
    wv = nc.declare_dram_parameter("wv", [E, DD], BF16, isOutput=False)
    bq = nc.declare_dram_parameter("bq", [DD, 1], F32, isOutput=False)
    w0 = nc.declare_dram_parameter("w0", [DD, O], BF16, isOutput=False)
    # partial outputs in bf16: halves the output DMA + psum->sbuf staging;
    # the 8 partials are summed in fp32 on host
    out = nc.declare_dram_parameter("out", [S, O], BF16, isOutput=True)

    EC = E // P               # 8 e-chunks
    SC = S // 512             # 4 s-chunks of 512
    TB = S // P               # 16 t-blocks
    NEX = 12                  # exp sbuf slots
    NOB = 4                   # output staging slots of [P, 1024]

    # ---- static SBUF allocation --------------------------------------
    xt_sb = nc.alloc_sbuf_tensor("xt_sb", [P, EC, S], BF16).ap()
    zt_sb = nc.alloc_sbuf_tensor("zt_sb", [P, EC, S], BF16).ap()
    # padded q: cols 0:512 head0 (rows 64:128 zero), 512:1024 head1 (rows 0:64 zero)
    qP_sb = nc.alloc_sbuf_tensor("qP_sb", [P, SC, 1024], BF16).ap()
    kT_sb = nc.alloc_sbuf_tensor("kT_sb", [P, S], BF16).ap()
    wq_sb = nc.alloc_sbuf_tensor("wq_sb", [P, EC, DD], BF16).ap()
    wk_sb = nc.alloc_sbuf_tensor("wk_sb", [P, EC, DD], BF16).ap()
    wv_sb = nc.alloc_sbuf_tensor("wv_sb", [P, EC, DD], BF16).ap()
    w0_sb = nc.alloc_sbuf_tensor("w0_sb", [P, O], BF16).ap()
    bq_sb = nc.alloc_sbuf_tensor("bq_sb", [P, 1], F32).ap()
    ones_row = nc.alloc_sbuf_tensor("ones_row", [1, 64], BF16).ap()
    vT_sb = nc.alloc_sbuf_tensor("vT_sb", [P, S], BF16).ap()
    ident = nc.alloc_sbuf_tensor("ident", [P, P], BF16).ap()
    v0_sb = nc.alloc_sbuf_tensor("v0_sb", [P, TB, 65], BF16).ap()
    v1_sb = nc.alloc_sbuf_tensor("v1_sb", [P, TB, 65], BF16).ap()
    ex_sb = nc.alloc_sbuf_tensor("ex_sb", [P, NEX, 1024], BF16).ap()
    E_sb = nc.alloc_sbuf_tensor("E_sb", [P, 2, 512], F32).ap()
    dmy_sb = nc.alloc_sbuf_tensor("dmy_sb", [P, 512], BF16).ap()
    rr_sb = nc.alloc_sbuf_tensor("rr_sb", [1, 2, 512], BF16).ap()
    lnt_sb = nc.alloc_sbuf_tensor("lnt_sb", [1, 2, 512], F32).ap()
    scr_sb = nc.alloc_sbuf_tensor("scr_sb", [1, 2], F32).ap()
    oT_sb = nc.alloc_sbuf_tensor("oT_sb", [P, 2, 512], BF16).ap()
    ob_sb = nc.alloc_sbuf_tensor("ob_sb", [P, NOB + 2, 1024], BF16).ap()

    # ---- static PSUM banks -------------------------------------------
    qa0 = nc.alloc_psum_tensor("qa0", [P, 1024], F32).ap()   # banks 0-1
    qa1 = nc.alloc_psum_tensor("qa1", [P, 1024], F32).ap()   # banks 2-3
    av0 = nc.alloc_psum_tensor("av0", [P, 512], F32).ap()    # bank 4
    av1 = nc.alloc_psum_tensor("av1", [P, 512], F32).ap()    # bank 5
    bcp = nc.alloc_psum_tensor("bcp", [P, 512], F32).ap()    # bank 6
    pjp = nc.alloc_psum_tensor("pjp", [P, 512], F32).ap()    # bank 7

    # ---- semaphores ---------------------------------------------------
    sQW = nc.alloc_semaphore("sQW")                          # wq+bq: 32
    sKW = nc.alloc_semaphore("sKW")                          # wk: 16
    sVW = nc.alloc_semaphore("sVW")                          # wv: 16
    sW0 = nc.alloc_semaphore("sW0")
    sXT = [nc.alloc_semaphore(f"sXT{c}") for c in range(4)]
    sZT = [nc.alloc_semaphore(f"sZT{c}") for c in range(4)]
    sOB = [nc.alloc_semaphore(f"sOB{j}") for j in range(NOB)]
    sOBX = [nc.alloc_semaphore(f"sOBX{j}") for j in range(2)]
    sGP = nc.alloc_semaphore("sGP")
    sLN = nc.alloc_semaphore("sLN")
    sWU = nc.alloc_semaphore("sWU")
    sQP = nc.alloc_semaphore("sQP")
    sPE = nc.alloc_semaphore("sPE")
    sACT = nc.alloc_semaphore("sACT")
    sDVE = nc.alloc_semaphore("sDVE")

    # ---- closed-form tick schedules ----------------------------------
    # PE ticks: q sc (4), k sc (4), v tb (16), then attention entries.
    def pe_q(sc):
        return sc + 1

    def pe_k(sc):
        return 4 + sc + 1

    def pe_vT(sc):
        return 8 + sc + 1

    def pe_tp(tb):
        return 12 + tb + 1

    def pe_scores(sc, tb):
        return PE_TICK[("scores", sc, tb)]

    def pe_av(sc, tb):
        return PE_TICK[("av", sc, tb)]

    def pe_bcast(sc, h):
        return PE_TICK[("bcast", sc, h)]

    def pe_proj(sc, sb, oc):
        return PE_TICK[("proj", sc, sb, oc)]

    # ACT: one tick per exp
    def act_exp(sc, tb):
        return sc * TB + tb + 1

    # DVE ticks: q sc (4), k sc (4), v tb (16), then per sc:
    # E-copy x2, recip x2, mult x2, ob x8 -> 14 per sc.
    def dve_q(sc):
        return sc + 1

    def dve_k(sc):
        return 4 + sc + 1

    def dve_vT(sc):
        return 8 + sc + 1

    def dve_vcopy(tb):
        return 12 + tb + 1

    def dve_ecp(sc, h):
        return 28 + sc * 14 + h + 1

    def dve_rcp(sc, h):
        return 28 + sc * 14 + 2 + h + 1

    def dve_mult(sc, h):
        return 28 + sc * 14 + 4 + h + 1

    def dve_ob(gi):
        sc, j = divmod(gi, 8)
        return 28 + sc * 14 + 6 + j + 1

    # software-pipelined PE attention order: scores run 2 iterations
    # ahead of AV.
    ATTN_ORD = [("scores", 0, 0), ("scores", 0, 1)]
    for sc_ in range(SC):
        for tb_ in range(TB):
            gn = sc_ * TB + tb_ + 2
            if gn < SC * TB:
                ATTN_ORD.append(("scores", gn // TB, gn % TB))
            ATTN_ORD.append(("av", sc_, tb_))
            # previous chunk's normalization-dependent PE work, deferred
            # deep enough that the ~4us reciprocals run off the critical path
            if sc_ > 0:
                pv = sc_ - 1
                for j_ in {6: [-1], 9: [-2], 10: [0], 11: [1, 2], 12: [3, 4],
                           13: [5, 6], 14: [7]}.get(tb_, []):
                    if j_ == -1:
                        ATTN_ORD.append(("bcast", pv, 0))
                    elif j_ == -2:
                        ATTN_ORD.append(("bcast", pv, 1))
                    else:
                        ATTN_ORD.append(("proj", pv, j_ // 2, j_ % 2))
    for h_ in range(2):
        ATTN_ORD.append(("bcast", SC - 1, h_))
    for sb_ in range(4):
        for oc_ in range(2):
            ATTN_ORD.append(("proj", SC - 1, sb_, oc_))
    PE_TICK = {e: 28 + i + 1 for i, e in enumerate(ATTN_ORD)}

    def ob_slot(di):
        if di == 13:
            return NOB
        if di == 15:
            return NOB + 1
        return di % NOB

    counts = {"PE": 0, "ACT": 0, "DVE": 0}

    def inc(eng, instr, sem, expect):
        instr.then_inc(sem, 1)
        counts[eng] += 1
        assert counts[eng] == expect, (eng, counts[eng], expect)

    class WaitTracker:
        def __init__(self, eng):
            self.eng = eng
            self.seen = {}

        def need(self, sem, val):
            if val <= 0:
                return
            key = sem.name
            if self.seen.get(key, -1) >= val:
                return
            self.seen[key] = val
            self.eng.wait_ge(sem, val)

    with nc.Block() as block:

        @block.sync
        def _(sp):
            sp.dma_start(out=wq_sb, in_=wq.rearrange("(p c) d -> p c d", p=P)).then_inc(sQW, 16)
            sp.dma_start(out=bq_sb, in_=bq[:, :]).then_inc(sQW, 16)
            xr = xT.rearrange("(p c) d -> p c d", p=P)
            for qi in range(4):
                sp.dma_start(out=xt_sb[:, 2 * qi:2 * qi + 2, :],
                             in_=xr[:, 2 * qi:2 * qi + 2, :]).then_inc(sXT[qi], 16)
            sp.dma_start(out=w0_sb, in_=w0[:, :]).then_inc(sW0, 16)
            w = WaitTracker(sp)
            for sc in range(SC):
                for sb in range(4):
                    di = sc * 4 + sb
                    if di % 2 != 0:
                        continue
                    row = sc * 512 + sb * P
                    w.need(sDVE, dve_ob(sc * 8 + 2 * sb + 1))
                    sp.dma_start(
                        out=out[row:row + P, :],
                        in_=ob_sb[:, ob_slot(di), :],
                    ).then_inc(sOB[di % NOB], 16)
            for j in range(NOB):
                nwrites = len([d for d in range(SC * 4) if d % NOB == j and ob_slot(d) == j])
                sp.wait_ge(sOB[j], 16 * nwrites)
            for j in range(2):
                sp.wait_ge(sOBX[j], 16)
            if os.environ.get("KDBG"):
                sDBG = nc.alloc_semaphore("sDBG")
                d_qP = nc.declare_dram_parameter("d_qP", [P, SC * 1024], mybir.dt.bfloat16, isOutput=True)
                d_kT = nc.declare_dram_parameter("d_kT", [P, S], mybir.dt.bfloat16, isOutput=True)
                d_v0 = nc.declare_dram_parameter("d_v0", [P, TB * 65], mybir.dt.bfloat16, isOutput=True)
                d_v1 = nc.declare_dram_parameter("d_v1", [P, TB * 65], mybir.dt.bfloat16, isOutput=True)
                d_ex = nc.declare_dram_parameter("d_ex", [P, NEX * 1024], mybir.dt.bfloat16, isOutput=True)
                d_rr = nc.declare_dram_parameter("d_rr", [1, 2 * 512], mybir.dt.bfloat16, isOutput=True)
                d_oT = nc.declare_dram_parameter("d_oT", [P, 2 * 512], mybir.dt.bfloat16, isOutput=True)
                sp.dma_start(out=d_qP[:, :], in_=qP_sb).then_inc(sDBG, 16)
                sp.dma_start(out=d_kT[:, :], in_=kT_sb).then_inc(sDBG, 16)
                sp.dma_start(out=d_v0[:, :], in_=v0_sb).then_inc(sDBG, 16)
                sp.dma_start(out=d_v1[:, :], in_=v1_sb).then_inc(sDBG, 16)
                sp.dma_start(out=d_ex[:, :], in_=ex_sb).then_inc(sDBG, 16)
                sp.dma_start(out=d_rr[:, :], in_=rr_sb).then_inc(sDBG, 16)
                sp.dma_start(out=d_oT[:, :], in_=oT_sb).then_inc(sDBG, 16)
                sp.wait_ge(sDBG, 16 * 7)

        @block.gpsimd
        def _(gp):
            gp.wait_ge(sGP, 1)
            from concourse.masks import make_identity
            make_identity(nc, ident, nomemset=True)
            nc.gpsimd.engine_nop().then_inc(sGP, 1)
            gw = WaitTracker(gp)
            for sc in range(SC):
                for sb in range(4):
                    di = sc * 4 + sb
                    if di % 2 != 1 or di in (13, 15):
                        continue
                    row = sc * 512 + sb * P
                    gw.need(sDVE, dve_ob(sc * 8 + 2 * sb + 1))
                    gp.dma_start(
                        out=out[row:row + P, :],
                        in_=ob_sb[:, ob_slot(di), :],
                    ).then_inc(sOB[di % NOB], 16)

        @block.tensor
        def _(pe):
            w = WaitTracker(pe)
            kbank = (av0, av1, bcp, pjp)
            # HAM warm-up: dummy matmuls keep the PE busy from t~0 so the
            # p-state reaches full clock before the real projections start.
            w.need(sWU, 1)
            for _wu in range(20):
                nc.tensor.matmul(
                    av0[:, :],
                    lhsT=dmy_sb[:, 0:128],
                    rhs=dmy_sb[:, :],
                    start=True, stop=True,
                    skip_group_check=True,
                )
            # Q (qa banks) and K (banks 4-7) interleaved per input quarter.
            for qi in range(4):
                w.need(sQW, 32)
                w.need(sXT[qi], 16)
                for e in range(2):
                    ec = 2 * qi + e
                    for sc in range(SC):
                        i = nc.tensor.matmul(
                            (qa0 if sc < 2 else qa1)[:, (sc % 2) * 512:(sc % 2) * 512 + 512],
                            lhsT=wq_sb[:, ec, :],
                            rhs=xt_sb[:, ec, sc * 512:(sc + 1) * 512],
                            start=(ec == 0), stop=(ec == EC - 1),
                            skip_group_check=True,
                        )
                        if ec == EC - 1:
                            inc("PE", i, sPE, pe_q(sc))
                w.need(sKW, 16)
                w.need(sZT[qi], 16)
                for e in range(2):
                    ec = 2 * qi + e
                    for sc in range(SC):
                        i = nc.tensor.matmul(
                            kbank[sc][:, :],
                            lhsT=wk_sb[:, ec, :],
                            rhs=zt_sb[:, ec, sc * 512:(sc + 1) * 512],
                            start=(ec == 0), stop=(ec == EC - 1),
                            skip_group_check=True,
                        )
                        if ec == EC - 1:
                            inc("PE", i, sPE, pe_k(sc))
            # V projection as vT [dd, t] into qa banks (after q drains),
            # chunk-at-a-time so each starts as soon as its q copy lands.
            for sc in range(SC):
                w.need(sVW, 16)
                w.need(sDVE, dve_q(sc))
                w.need(sQP, sc + 1)
                for ec in range(EC):
                    i = nc.tensor.matmul(
                        (qa0 if sc < 2 else qa1)[:, (sc % 2) * 512:(sc % 2) * 512 + 512],
                        lhsT=wv_sb[:, ec, :],
                        rhs=zt_sb[:, ec, sc * 512:(sc + 1) * 512],
                        start=(ec == 0), stop=(ec == EC - 1),
                        skip_group_check=True,
                    )
                    if ec == EC - 1:
                        inc("PE", i, sPE, pe_vT(sc))
            w.need(sGP, 2)
            for tb in range(TB):
                tgt = (bcp if tb % 2 == 0 else pjp)[0:P, 0:64].bitcast(BF16)
                w.need(sDVE, dve_vT(tb // 4))
                w.need(sDVE, dve_k(2 if tb % 2 == 0 else 3))
                if tb >= 2:
                    w.need(sDVE, dve_vcopy(tb - 2))
                i = nc.tensor.transpose(tgt, vT_sb[:, tb * P:(tb + 1) * P], ident)
                inc("PE", i, sPE, pe_tp(tb))
            # Attention + projection, software-pipelined.
            for ent in ATTN_ORD:
                kind = ent[0]
                if kind == "scores":
                    _, sc, tb = ent
                    g = sc * TB + tb
                    qa = qa0 if tb % 2 == 0 else qa1
                    w.need(sDVE, dve_q(sc))
                    w.need(sQP, sc + 1)
                    w.need(sDVE, dve_k(tb // 4))
                    # qa bank pair was last read by the vT copies of the
                    # two projection chunks it held
                    w.need(sDVE, dve_vT(1 if tb % 2 == 0 else 3))
                    if g >= 2:
                        w.need(sACT, g - 1)
                    # one shared kT stationary; two 512-wide streams (PSUM
                    # matmul output is limited to one bank)
                    nc.tensor.matmul(
                        qa[:, 0:512],
                        lhsT=kT_sb[:, tb * P:(tb + 1) * P],
                        rhs=qP_sb[:, sc, 0:512],
                        start=True, stop=True,
                    )
                    i = nc.tensor.matmul(
                        qa[:, 512:1024],
                        lhsT=kT_sb[:, tb * P:(tb + 1) * P],
                        rhs=qP_sb[:, sc, 512:1024],
                        start=True, stop=True,
                    )
                    inc("PE", i, sPE, pe_scores(sc, tb))
                elif kind == "av":
                    _, sc, tb = ent
                    g = sc * TB + tb
                    if tb == 0 and sc > 0:
                        w.need(sDVE, dve_ecp(sc - 1, 1))
                    if tb == 0 and sc == 0:
                        w.need(sDVE, dve_k(1))
                    w.need(sDVE, dve_vcopy(tb))
                    w.need(sACT, act_exp(sc, tb))
                    slot = g % NEX
                    nc.tensor.matmul(
                        av0[0:65, :],
                        lhsT=v0_sb[:, tb, :],
                        rhs=ex_sb[:, slot, 0:512],
                        start=(tb == 0), stop=(tb == TB - 1),
                        skip_group_check=True,
                    )
                    i = nc.tensor.matmul(
                        av1[0:65, :],
                        lhsT=v1_sb[:, tb, :],
                        rhs=ex_sb[:, slot, 512:1024],
                        start=(tb == 0), stop=(tb == TB - 1),
                        skip_group_check=True,
                    )
                    inc("PE", i, sPE, pe_av(sc, tb))
                elif kind == "bcast":
                    _, sc, h = ent
                    if sc == SC - 1:
                        w.need(sACT, SC * TB + h + 1)
                    else:
                        w.need(sDVE, dve_rcp(sc, h))
                    if h == 1:
                        w.need(sDVE, dve_mult(sc, 0))
                    elif sc > 0:
                        w.need(sDVE, dve_ob((sc - 1) * 8 + 7))
                    i = nc.tensor.matmul(
                        bcp[0:64, :],
                        lhsT=ones_row[0:1, :],
                        rhs=rr_sb[0:1, h, :],
                        start=True, stop=True,
                    )
                    inc("PE", i, sPE, pe_bcast(sc, h))
                else:
                    _, sc, sb, oc = ent
                    gi = sc * 8 + sb * 2 + oc
                    j = sb * 2 + oc
                    if sc == SC - 1:
                        # scores are done: qa banks are free, so the tail
                        # projections fan out and do not wait on the ob ladder
                        bank = (qa0[:, 0:512], qa0[:, 512:1024],
                                qa1[:, 0:512], qa1[:, 512:1024],
                                pjp[:, :], bcp[:, :])[j % 6]
                        if j >= 6:
                            w.need(sDVE, dve_ob(sc * 8 + j - 6))
                    else:
                        bank = (pjp if gi % 2 == 0 else bcp)[:, :]
                        if gi >= 2:
                            w.need(sDVE, dve_ob(gi - 2))
                    w.need(sW0, 16)
                    w.need(sDVE, dve_mult(sc, 1))
                    i = nc.tensor.matmul(
                        bank,
                        lhsT=oT_sb[:, sc % 2, sb * P:(sb + 1) * P],
                        rhs=w0_sb[:, oc * 512:(oc + 1) * 512],
                        start=True, stop=True,
                    )
                    inc("PE", i, sPE, pe_proj(sc, sb, oc))

        @block.scalar
        def _(act):
            w = WaitTracker(act)
            zr = zT.rearrange("(p c) d -> p c d", p=P)
            act.dma_start(out=wk_sb, in_=wk.rearrange("(p c) d -> p c d", p=P)).then_inc(sKW, 16)
            act.dma_start(out=wv_sb, in_=wv.rearrange("(p c) d -> p c d", p=P)).then_inc(sVW, 16)
            for qi in range(4):
                act.dma_start(out=zt_sb[:, 2 * qi:2 * qi + 2, :],
                              in_=zr[:, 2 * qi:2 * qi + 2, :]).then_inc(sZT[qi], 16)
            Ident = mybir.ActivationFunctionType.Identity
            for sc in range(SC):
                w.need(sQW, 32)
                w.need(sPE, pe_q(sc))
                qa = (qa0 if sc < 2 else qa1)[:, (sc % 2) * 512:(sc % 2) * 512 + 512]
                nc.scalar.activation(
                    qP_sb[64:P, sc, 512:1024], qa[64:P, :], Ident,
                    bias=bq_sb[64:P, 0:1]).then_inc(sQP, 1)
            for sc in range(SC):
                for tb in range(TB):
                    g = sc * TB + tb
                    w.need(sPE, pe_scores(sc, tb))
                    if g >= NEX:
                        gp_sc, gp_tb = divmod(g - NEX, TB)
                        w.need(sPE, pe_av(gp_sc, gp_tb))
                    slot = g % NEX
                    qa = qa0 if tb % 2 == 0 else qa1
                    i = nc.scalar.activation(
                        ex_sb[:, slot, :], qa[:, :], Exp, scale=0.125)
                    inc("ACT", i, sACT, act_exp(sc, tb))
            # last chunk's reciprocal via Ln+Exp (ACT is idle by then), so the
            # tail does not pay the ~4us DVE reciprocal
            for h in range(2):
                w.need(sDVE, dve_ecp(SC - 1, h))
                nc.scalar.activation(lnt_sb[0:1, h, :], E_sb[64:65, h, :],
                                     mybir.ActivationFunctionType.Ln).then_inc(sLN, 1)
                w.need(sLN, h + 1)
                i = nc.scalar.activation(rr_sb[0:1, h, :], lnt_sb[0:1, h, :],
                                         Exp, scale=-1.0)
                inc("ACT", i, sACT, SC * TB + h + 1)
            for xi, di in enumerate((13, 15)):
                sb = di % 4
                row = (SC - 1) * 512 + sb * P
                w.need(sDVE, dve_ob((SC - 1) * 8 + 2 * sb + 1))
                act.dma_start(
                    out=out[row:row + P, :],
                    in_=ob_sb[:, ob_slot(di), :],
                ).then_inc(sOBX[xi], 16)

        @block.vector
        def _(dve):
            w = WaitTracker(dve)
            # zero the q pads once; later ticks imply completion (in-order)
            nc.vector.memset(dmy_sb, 0.0).then_inc(sWU, 1)
            nc.vector.memset(ident, 0.0).then_inc(sGP, 1)
            nc.vector.memset(qP_sb[64:P, :, 0:512], 0.0)
            nc.vector.memset(qP_sb[0:64, :, 512:1024], 0.0)
            nc.vector.memset(ones_row, 1.0)
            nc.vector.memset(v0_sb[:, :, 64:65], 1.0)
            nc.vector.memset(v1_sb[:, :, 64:65], 1.0)
            # q: bias-add + cast into padded layout (head0 half; head1 on ACT)
            for sc in range(SC):
                w.need(sQW, 32)
                w.need(sPE, pe_q(sc))
                qa = (qa0 if sc < 2 else qa1)[:, (sc % 2) * 512:(sc % 2) * 512 + 512]
                i = nc.vector.tensor_scalar_add(
                    out=qP_sb[0:64, sc, 0:512],
                    in0=qa[0:64, :],
                    scalar1=bq_sb[0:64, 0:1],
                )
                inc("DVE", i, sDVE, dve_q(sc))
            # k: plain cast copy out of banks 4-7
            kbank = (av0, av1, bcp, pjp)
            for sc in range(SC):
                w.need(sPE, pe_k(sc))
                i = nc.vector.tensor_copy(kT_sb[:, sc * 512:(sc + 1) * 512],
                                          kbank[sc][:, :])
                inc("DVE", i, sDVE, dve_k(sc))
            # vT: cast copy out of qa banks
            for sc in range(SC):
                w.need(sPE, pe_vT(sc))
                qa = (qa0 if sc < 2 else qa1)[:, (sc % 2) * 512:(sc % 2) * 512 + 512]
                i = nc.vector.tensor_copy(vT_sb[:, sc * 512:(sc + 1) * 512], qa)
                inc("DVE", i, sDVE, dve_vT(sc))
            # v: split transposed [t, dd] blocks into per-head [t, 64] slots
            for tb in range(TB):
                src = (bcp if tb % 2 == 0 else pjp)[0:P, 0:64].bitcast(BF16)
                w.need(sPE, pe_tp(tb))
                nc.vector.tensor_copy(v0_sb[:, tb, 0:64], src[:, 0:64])
                i = nc.vector.tensor_copy(v1_sb[:, tb, 0:64], src[:, 64:128])
                inc("DVE", i, sDVE, dve_vcopy(tb))
            # attention normalization + output staging
            for sc in range(SC):
                for h, av in ((0, av0), (1, av1)):
                    w.need(sPE, pe_av(sc, TB - 1))
                    i = nc.vector.tensor_copy(E_sb[0:65, h, :], av[0:65, :])
                    inc("DVE", i, sDVE, dve_ecp(sc, h))
                for h in range(2):
                    w.need(sDVE, dve_ecp(sc, h))
                    if sc == SC - 1:
                        i = nc.vector.memset(scr_sb[0:1, h:h + 1], 0.0)
                    else:
                        i = nc.vector.reciprocal(rr_sb[0:1, h, :],
                                                 E_sb[64:65, h, :])
                    inc("DVE", i, sDVE, dve_rcp(sc, h))
                for h in range(2):
                    w.need(sPE, pe_bcast(sc, h))
                    i = nc.vector.tensor_mul(
                        oT_sb[h * 64:(h + 1) * 64, sc % 2, :],
                        E_sb[0:64, h, :], bcp[0:64, :])
                    inc("DVE", i, sDVE, dve_mult(sc, h))
                for j in range(8):
                    sb, oc = divmod(j, 2)
                    gi = sc * 8 + j
                    di = sc * 4 + sb
                    if sc == SC - 1:
                        bank = (qa0[:, 0:512], qa0[:, 512:1024],
                                qa1[:, 0:512], qa1[:, 512:1024],
                                pjp[:, :], bcp[:, :])[j % 6]
                    else:
                        bank = (pjp if gi % 2 == 0 else bcp)[:, :]
                    w.need(sPE, pe_proj(sc, sb, oc))
                    if di >= NOB and ob_slot(di) == di % NOB:
                        w.need(sOB[di % NOB], 16 * (di // NOB - (1 if di in (13 + NOB, 15 + NOB) else 0)))
                    i = nc.vector.tensor_copy(
                        ob_sb[:, ob_slot(di), oc * 512:(oc + 1) * 512], bank)
                    inc("DVE", i, sDVE, dve_ob(gi))

    _lp.close()
    return nc


def _get_nc():
    if "nc" not in _built:
        _built["nc"] = _build_bass()
    return _built["nc"]


def _make_in_maps(x, z, Wq, bq, Wk, Wv, W0):
    import concourse.mybir as mybir
    BF = mybir.dt.np(mybir.dt.bfloat16)
    xT = np.ascontiguousarray(x.T).astype(BF)
    zT = np.ascontiguousarray(z.T).astype(BF)
    in_maps = []
    for c in range(NCORES):
        h0, h1 = 2 * c, 2 * c + 1
        in_maps.append({
            "xT": xT,
            "zT": zT,
            "wq": np.ascontiguousarray(np.concatenate([Wq[h0], Wq[h1]], axis=1)).astype(BF),
            "wk": np.ascontiguousarray(np.concatenate([Wk[h0], Wk[h1]], axis=1)).astype(BF),
            "wv": np.ascontiguousarray(np.concatenate([Wv[h0], Wv[h1]], axis=1)).astype(BF),
            "bq": np.ascontiguousarray(np.concatenate([bq[h0], bq[h1]]).reshape(DD, 1), np.float32),
            "w0": np.ascontiguousarray(W0[c * DD:(c + 1) * DD, :]).astype(BF),
        })
    return in_maps


def _numpy_reference(x, z, mask, Wq, bq, Wk, bk, Wv, bv, W0, b0):
    # general-mask fallback (not the benchmarked path; harness mask is all-ones)
    x = x.astype(np.float64); z = z.astype(np.float64)
    q = np.einsum("se,hed->hsd", x, Wq) + bq[:, None, :]
    k = np.einsum("te,hed->htd", z, Wk) + bk[:, None, :]
    v = np.einsum("te,hem->htm", z, Wv) + bv[:, None, :]
    s = np.einsum("hsd,htd->hst", q, k) / np.sqrt(np.float64(D))
    s = np.where(mask[None, :, :] == 0, -np.inf, s)
    s = s - s.max(axis=-1, keepdims=True)
    e = np.exp(s)
    a = e / e.sum(axis=-1, keepdims=True)
    o = np.einsum("hst,htm->hsm", a, v)
    o = np.transpose(o, (1, 0, 2)).reshape(S, H * MD)
    return (o @ W0 + b0).astype(np.float32)


def kernel(x, z, mask, Wq, bq, Wk, bk, Wv, bv, W0, b0):
    global LAST_EXEC_TIME_NS, LAST_RESULTS
    arrs = {k: np.asarray(v) for k, v in dict(
        x=x, z=z, mask=mask, Wq=Wq, bq=bq, Wk=Wk, bk=bk, Wv=Wv, bv=bv,
        W0=W0, b0=b0).items()}
    if not bool((arrs["mask"] != 0).all()):
        return _numpy_reference(**arrs)

    from concourse.bass_utils import run_bass_kernel_spmd

    nc = _get_nc()
    in_maps = _make_in_maps(
        arrs["x"], arrs["z"], arrs["Wq"], arrs["bq"], arrs["Wk"],
        arrs["Wv"], arrs["W0"])
    trace = bool(os.environ.get("KERNEL_TRACE"))
    kw = {}
    td = os.environ.get("KERNEL_TRACE_DIR")
    if td:
        os.makedirs(td, exist_ok=True)
        kw["tmpdir"] = td
    res = run_bass_kernel_spmd(
        nc, in_maps, core_ids=list(range(NCORES)), trace=trace, **kw
    )
    LAST_EXEC_TIME_NS = res.exec_time_ns
    LAST_RESULTS = res
    acc = np.zeros((S, O), dtype=np.float32)
    for rm in res.results:
        acc += rm["out"].astype(np.float32)
    # bv is not applied on-device: sum_t softmax * bv == bv, so it folds
    # into the final bias through W0.
    b0p = (arrs["b0"].astype(np.float64)
           + arrs["bv"].reshape(-1).astype(np.float64) @ arrs["W0"].astype(np.float64))
    acc += b0p.astype(np.float32)[None, :]
    return acc

